# revision 18
# baseline (speedup 1.0000x reference)
"""ClassAwareTripletLoss Trainium2 kernel (8 NeuronCores).

Only anchors with label w=1 contribute to the loss (tri * w), so the host
compacts the valid (sample, class) anchor rows GLOBALLY and block-partitions
them across the 8 cores (32640 of 65536 rows survive -> 32 tiles of 128 per
core instead of 64, perfectly load-balanced). The host pre-transposes and
casts operands to bf16, so the device does exactly the O(bs*C*C*D) work:

  per pair of tiles: 4 matmuls  xT (stationary, the pair row-packed into
  partition halves -> concurrent PE row-groups) @ protT -> two PSUM units
  of [128, 1024] raw dots sharing one [128, 2, 1024] pool tile (4 banks).
  per pair: one drain on one engine (pairs alternate so both engines run):
     - VectorE: single fused tensor_reduce [128, 2, 1024] -> [128, 2]
                (true max, self-class kept: P ~ 1/1024 per row)
     - ScalarE: 2x Exp activation + accumulate (LSE: max ~= (ln(acc)+
                RSHIFT)/RSCALE; the self-class term is subtracted on host
                using the exact dot). Exp table pre-warmed at t=0 so the
                ~2.7us ACT_TABLE_LOAD overlaps the input DMAs.
  PSUM (8 banks) holds exactly 2 pair tiles -> matmuls double-buffer
  against drains. The [128, 32] result is DMA'd out; the tiny [bs, C]
  epilogue (normalize, sqrt, relu, per-sample mean) runs on host in f64.

GPSIMD cannot access PSUM on TRN2 and DMA cannot read PSUM, so DVE + ACT
are the only drain engines; the split N_DVE_PAIRS tunes their balance.
Raw dots are used (x not normalized on device); the host divides by ||x||.
RSCALE/RSHIFT follow the nominal ||x|| ~ sqrt(D) = 8 so the effective LSE
beta on normalized dots is ~100 (bias < ~1e-3 on the max).
"""

import numpy as np
import ml_dtypes
from contextlib import ExitStack

import concourse.bass as bass  # noqa: F401  (side-effect imports)
import concourse.bacc as bacc
import concourse.tile as tile
from concourse import mybir
from concourse.bass_utils import run_bass_kernel_spmd

f32 = mybir.dt.float32
bf16 = mybir.dt.bfloat16
AF = mybir.ActivationFunctionType
X = mybir.AxisListType.X

BS, C, D = 64, 1024, 64
NCORES = 8
RSCALE = 100.0 / 8.0   # LSE scale on raw dots (nominal ||x|| = 8)
RSHIFT = 35.0          # recentering so exp/acc stay in range
MARGIN = 0.2

# drain split, per unit: DVE reduce_max vs ACT LSE (interleaved)
DVE_UNIT_FRAC = 17 / 32


def unit_assignment(nt):
    n_d = int(round(nt * DVE_UNIT_FRAC))
    n_a = nt - n_d
    w = {"D": max(n_d, 1), "A": max(n_a, 1)}
    load = {"D": 0, "A": 0}
    seq = []
    for _ in range(nt):
        e = min("DA", key=lambda k: (load[k] + 1) / w[k])
        load[e] += 1
        seq.append(e)
    return seq


def build(nt):
    assert nt % 2 == 0
    npairs = nt // 2
    eng = unit_assignment(nt)

    nc = bacc.Bacc("TRN2", target_bir_lowering=False, debug=False)
    xT2_d = nc.dram_tensor("xT2", [128, npairs * 128], bf16,
                           kind="ExternalInput")
    pT2_d = nc.dram_tensor("protT2", [128, C], bf16, kind="ExternalInput")
    out_d = nc.dram_tensor("out", [128, nt], f32, kind="ExternalOutput")

    with tile.TileContext(nc) as tc, ExitStack() as ctx:
        P = ctx.enter_context(tc.tile_pool(name="persist", bufs=1))
        scrp = ctx.enter_context(tc.tile_pool(name="scr", bufs=2))
        ps = ctx.enter_context(tc.tile_pool(name="ps", bufs=4, space="PSUM"))

        nbeta = P.tile([128, 1], f32, tag="nbeta")
        nc.vector.memset(nbeta, -RSHIFT)

        pT2 = P.tile([128, C], bf16, tag="pT2")

        # input DMAs: first-needed data (chunk 0 + prototypes) first in
        # program order on the ACT ring, the rest on the Sync ring; one
        # SBUF tile per chunk so a pair's LDWEIGHTS waits only on its own
        # chunk's DMA (tile-granular dependency tracking)
        if npairs >= 8:
            chunks = [(0, 2, nc.scalar), (2, 8, nc.sync),
                      (8, 12, nc.sync), (12, npairs, nc.sync)]
        else:
            chunks = [(0, npairs, nc.scalar)]
        xtiles = [None] * npairs
        first = True
        for a, b, engine in chunks:
            xc = P.tile([128, b - a, 128], bf16, tag=f"xc{a}")
            engine.dma_start(
                out=xc,
                in_=xT2_d.ap()[:, a * 128:b * 128].rearrange(
                    "p (q c) -> p q c", c=128))
            if first:
                nc.scalar.dma_start(out=pT2, in_=pT2_d.ap())
                first = False
            for p in range(a, b):
                xtiles[p] = xc[:, p - a, :]

        # pre-warm the Exp table so ACT_TABLE_LOAD overlaps the input DMAs
        warm = P.tile([128, 1], bf16, tag="warm")
        nc.scalar.activation(warm, nbeta, AF.Exp)

        out_sb = P.tile([128, nt], f32, tag="out_sb")

        # unit-granular PSUM tiles: 4 units in flight (8 banks), so both
        # drain engines run concurrently while the PE fills the next two
        for u in range(nt):
            p, h2 = divmod(u, 2)
            pst = ps.tile([128, 2, 512], f32, tag="psu")
            lhsT = xtiles[p][64 * h2:64 * (h2 + 1), :]
            for h in range(2):
                rhs = pT2[64 * h2:64 * (h2 + 1), h * 512:(h + 1) * 512]
                nc.tensor.matmul(pst[:, h, :], lhsT, rhs,
                                 start=True, stop=True)
            flat = pst.rearrange("p a n -> p (a n)")
            if eng[u] == "D":
                nc.vector.reduce_max(out=out_sb[:, u:u + 1], in_=flat,
                                     axis=X)
            else:
                scr = scrp.tile([128, 1024], bf16, tag="scr")
                nc.scalar.activation(scr, flat, AF.Exp,
                                     bias=nbeta, scale=RSCALE,
                                     accum_out=out_sb[:, u:u + 1])

        # ship the bulk of the result as soon as those drains finish; only
        # the last few columns wait for the tail drains
        cut = max(0, nt - 6)
        if cut:
            nc.sync.dma_start(out=out_d.ap()[:, :cut], in_=out_sb[:, :cut])
        nc.sync.dma_start(out=out_d.ap()[:, cut:], in_=out_sb[:, cut:])

    nc.compile()
    return nc, eng


_NC = {}


def _get_nc(nt):
    if nt not in _NC:
        _NC[nt] = build(nt)
    return _NC[nt]


def _prep(inputs, label, pos_prot):
    """Host-side global compaction + operand prep."""
    inputs = np.asarray(inputs, np.float32)
    lab = np.asarray(label, np.float32)[:, :, 0]
    prot = np.asarray(pos_prot, np.float32)

    b_all, c_all = np.nonzero(lab > 0.5)
    nv_tot = len(b_all)
    per_core = -(-nv_tot // NCORES)
    nt = max(2, 2 * ((-(-per_core // 128) + 1) // 2))
    n = nt * 128
    npairs = nt // 2

    protT2 = np.concatenate([prot.T, prot.T], axis=0)  # [128, 1024]
    protT2 = protT2.astype(ml_dtypes.bfloat16)

    in_maps = []
    meta = []
    for i in range(NCORES):
        sl = slice(i * per_core, min((i + 1) * per_core, nv_tot))
        b_idx, c_idx = b_all[sl], c_all[sl]
        nv = len(b_idx)
        xr = np.zeros((n, D), np.float32)
        xr[:nv] = inputs[b_idx, c_idx]
        nrm = np.linalg.norm(xr[:nv].astype(np.float64), axis=1)
        invn = 1.0 / np.maximum(nrm, 1e-12)
        dd = np.einsum("nd,nd->n", xr[:nv].astype(np.float64),
                       prot[c_idx].astype(np.float64))
        # [p, h2, j, d] -> partition (h2*64+d), column (p*128+j)
        xT2 = xr.reshape(npairs, 2, 128, D).transpose(1, 3, 0, 2)
        xT2 = np.ascontiguousarray(xT2.reshape(128, npairs * 128))
        in_maps.append({
            "xT2": xT2.astype(ml_dtypes.bfloat16),
            "protT2": protT2,
        })
        meta.append((b_idx, c_idx, invn, dd))
    return nt, in_maps, meta


def _finish(res, nt, meta, eng):
    n = nt * 128
    per_sample_num = np.zeros(BS)
    per_sample_den = np.zeros(BS)
    for i in range(NCORES):
        b_idx, c_idx, invn, dd = meta[i]
        nv = len(b_idx)
        out = np.asarray(res.results[i]["out"], np.float64)  # [128, nt]
        m_raw = out.T.reshape(n)[:nv].copy()                 # row u*128+j
        isA = np.zeros(n, bool)
        for u in range(nt):
            if eng[u] == "A":
                isA[u * 128:(u + 1) * 128] = True
        isA = isA[:nv]
        acc = m_raw[isA] - np.exp(RSCALE * dd[isA] - RSHIFT)
        m_raw[isA] = (np.log(np.maximum(acc, 1e-30)) + RSHIFT) / RSCALE
        md = m_raw * invn
        ddn = dd * invn
        d_an = np.sqrt(np.maximum(2.0 - 2.0 * md, 0.0))
        d_ap = np.sqrt(np.maximum(2.0 - 2.0 * ddn, 0.0))
        tri = np.maximum(d_ap - d_an + MARGIN, 0.0)
        np.add.at(per_sample_num, b_idx, tri)
        np.add.at(per_sample_den, b_idx, 1.0)
    return np.float32(np.mean(per_sample_num / per_sample_den))


def run_cores(inputs, label, pos_prot, trace=False, tmpdir=None):
    nt, in_maps, meta = _prep(inputs, label, pos_prot)
    nc, eng = _get_nc(nt)
    kw = {}
    if trace:
        kw = dict(trace=True, tmpdir=tmpdir)
    res = run_bass_kernel_spmd(nc, in_maps, core_ids=list(range(NCORES)), **kw)
    return res, nt, meta, eng


def kernel(inputs, label, pos_prot, only_update=0, **_unused):
    res, nt, meta, eng = run_cores(np.asarray(inputs), np.asarray(label),
                                   np.asarray(pos_prot))
    return _finish(res, nt, meta, eng)


# revision 20
# speedup vs baseline: 1.0424x; 1.0424x over previous
"""ClassAwareTripletLoss Trainium2 kernel (8 NeuronCores).

Only anchors with label w=1 contribute to the loss (tri * w), so the host
compacts the valid (sample, class) anchor rows GLOBALLY and block-partitions
them across the 8 cores (32640 of 65536 rows survive -> 32 tiles of 128 per
core instead of 64, perfectly load-balanced). The host pre-transposes and
casts operands to bf16, so the device does exactly the O(bs*C*C*D) work:

  per pair of tiles: 4 matmuls  xT (stationary, the pair row-packed into
  partition halves -> concurrent PE row-groups) @ protT -> two PSUM units
  of [128, 1024] raw dots sharing one [128, 2, 1024] pool tile (4 banks).
  per pair: one drain on one engine (pairs alternate so both engines run):
     - VectorE: single fused tensor_reduce [128, 2, 1024] -> [128, 2]
                (true max, self-class kept: P ~ 1/1024 per row)
     - ScalarE: 2x Exp activation + accumulate (LSE: max ~= (ln(acc)+
                RSHIFT)/RSCALE; the self-class term is subtracted on host
                using the exact dot). Exp table pre-warmed at t=0 so the
                ~2.7us ACT_TABLE_LOAD overlaps the input DMAs.
  PSUM (8 banks) holds exactly 2 pair tiles -> matmuls double-buffer
  against drains. The [128, 32] result is DMA'd out; the tiny [bs, C]
  epilogue (normalize, sqrt, relu, per-sample mean) runs on host in f64.

GPSIMD cannot access PSUM on TRN2 and DMA cannot read PSUM, so DVE + ACT
are the only drain engines; the split N_DVE_PAIRS tunes their balance.
Raw dots are used (x not normalized on device); the host divides by ||x||.
RSCALE/RSHIFT follow the nominal ||x|| ~ sqrt(D) = 8 so the effective LSE
beta on normalized dots is ~100 (bias < ~1e-3 on the max).
"""

import numpy as np
import ml_dtypes
from contextlib import ExitStack

import concourse.bass as bass  # noqa: F401  (side-effect imports)
import concourse.bacc as bacc
import concourse.tile as tile
from concourse import mybir
from concourse.bass_utils import run_bass_kernel_spmd

f32 = mybir.dt.float32
bf16 = mybir.dt.bfloat16
AF = mybir.ActivationFunctionType
X = mybir.AxisListType.X

BS, C, D = 64, 1024, 64
NCORES = 8
RSCALE = 100.0 / 8.0   # LSE scale on raw dots (nominal ||x|| = 8)
RSHIFT = 35.0          # recentering so exp/acc stay in range
MARGIN = 0.2

# drain split, per unit: DVE reduce_max vs ACT LSE (interleaved)
DVE_UNIT_FRAC = 17 / 32


def unit_assignment(nt):
    n_d = int(round(nt * DVE_UNIT_FRAC))
    n_a = nt - n_d
    w = {"D": max(n_d, 1), "A": max(n_a, 1)}
    load = {"D": 0, "A": 0}
    seq = []
    for _ in range(nt):
        e = min("DA", key=lambda k: (load[k] + 1) / w[k])
        load[e] += 1
        seq.append(e)
    return seq


def build(nt):
    assert nt % 2 == 0
    npairs = nt // 2
    eng = unit_assignment(nt)

    nc = bacc.Bacc("TRN2", target_bir_lowering=False, debug=False)
    xT2_d = nc.dram_tensor("xT2", [128, npairs * 128], bf16,
                           kind="ExternalInput")
    pT2_d = nc.dram_tensor("protT2", [128, C], bf16, kind="ExternalInput")
    out_d = nc.dram_tensor("out", [128, nt], f32, kind="ExternalOutput")

    with tile.TileContext(nc) as tc, ExitStack() as ctx:
        P = ctx.enter_context(tc.tile_pool(name="persist", bufs=1))
        scrp = ctx.enter_context(tc.tile_pool(name="scr", bufs=2))
        ps = ctx.enter_context(tc.tile_pool(name="ps", bufs=4, space="PSUM"))

        nbeta = P.tile([128, 1], f32, tag="nbeta")
        nc.vector.memset(nbeta, -RSHIFT)

        pT2 = P.tile([128, C], bf16, tag="pT2")

        # input DMAs split across the two HWDGE queues (Sync + ACT), one
        # SBUF tile per chunk so a pair's LDWEIGHTS waits only on its own
        # chunk's DMA (tile-granular dependency tracking)
        if npairs >= 8:
            chunks = [(0, 2, nc.sync), (2, 8, nc.scalar),
                      (8, 12, nc.sync), (12, npairs, nc.sync)]
        else:
            chunks = [(0, npairs, nc.sync)]
        nc.sync.dma_start(out=pT2, in_=pT2_d.ap())
        xtiles = [None] * npairs
        for a, b, engine in chunks:
            xc = P.tile([128, b - a, 128], bf16, tag=f"xc{a}")
            engine.dma_start(
                out=xc,
                in_=xT2_d.ap()[:, a * 128:b * 128].rearrange(
                    "p (q c) -> p q c", c=128))
            for p in range(a, b):
                xtiles[p] = xc[:, p - a, :]

        # pre-warm the Exp table so ACT_TABLE_LOAD overlaps the input DMAs
        warm = P.tile([128, 1], bf16, tag="warm")
        nc.scalar.activation(warm, nbeta, AF.Exp)

        out_sb = P.tile([128, nt], f32, tag="out_sb")

        # unit-granular PSUM tiles: 4 units in flight (8 banks), so both
        # drain engines run concurrently while the PE fills the next two
        for u in range(nt):
            p, h2 = divmod(u, 2)
            pst = ps.tile([128, 2, 512], f32, tag="psu")
            lhsT = xtiles[p][64 * h2:64 * (h2 + 1), :]
            for h in range(2):
                rhs = pT2[64 * h2:64 * (h2 + 1), h * 512:(h + 1) * 512]
                nc.tensor.matmul(pst[:, h, :], lhsT, rhs,
                                 start=True, stop=True)
            flat = pst.rearrange("p a n -> p (a n)")
            if eng[u] == "D":
                nc.vector.reduce_max(out=out_sb[:, u:u + 1], in_=flat,
                                     axis=X)
            else:
                scr = scrp.tile([128, 1024], bf16, tag="scr")
                nc.scalar.activation(scr, flat, AF.Exp,
                                     bias=nbeta, scale=RSCALE,
                                     accum_out=out_sb[:, u:u + 1])

        nc.sync.dma_start(out=out_d.ap(), in_=out_sb)

    nc.compile()
    return nc, eng


_NC = {}


def _get_nc(nt):
    if nt not in _NC:
        _NC[nt] = build(nt)
    return _NC[nt]


def _prep(inputs, label, pos_prot):
    """Host-side global compaction + operand prep."""
    inputs = np.asarray(inputs, np.float32)
    lab = np.asarray(label, np.float32)[:, :, 0]
    prot = np.asarray(pos_prot, np.float32)

    b_all, c_all = np.nonzero(lab > 0.5)
    nv_tot = len(b_all)
    per_core = -(-nv_tot // NCORES)
    nt = max(2, 2 * ((-(-per_core // 128) + 1) // 2))
    n = nt * 128
    npairs = nt // 2

    protT2 = np.concatenate([prot.T, prot.T], axis=0)  # [128, 1024]
    protT2 = protT2.astype(ml_dtypes.bfloat16)

    in_maps = []
    meta = []
    for i in range(NCORES):
        sl = slice(i * per_core, min((i + 1) * per_core, nv_tot))
        b_idx, c_idx = b_all[sl], c_all[sl]
        nv = len(b_idx)
        xr = np.zeros((n, D), np.float32)
        xr[:nv] = inputs[b_idx, c_idx]
        nrm = np.linalg.norm(xr[:nv].astype(np.float64), axis=1)
        invn = 1.0 / np.maximum(nrm, 1e-12)
        dd = np.einsum("nd,nd->n", xr[:nv].astype(np.float64),
                       prot[c_idx].astype(np.float64))
        # [p, h2, j, d] -> partition (h2*64+d), column (p*128+j)
        xT2 = xr.reshape(npairs, 2, 128, D).transpose(1, 3, 0, 2)
        xT2 = np.ascontiguousarray(xT2.reshape(128, npairs * 128))
        in_maps.append({
            "xT2": xT2.astype(ml_dtypes.bfloat16),
            "protT2": protT2,
        })
        meta.append((b_idx, c_idx, invn, dd))
    return nt, in_maps, meta


def _finish(res, nt, meta, eng):
    n = nt * 128
    per_sample_num = np.zeros(BS)
    per_sample_den = np.zeros(BS)
    for i in range(NCORES):
        b_idx, c_idx, invn, dd = meta[i]
        nv = len(b_idx)
        out = np.asarray(res.results[i]["out"], np.float64)  # [128, nt]
        m_raw = out.T.reshape(n)[:nv].copy()                 # row u*128+j
        isA = np.zeros(n, bool)
        for u in range(nt):
            if eng[u] == "A":
                isA[u * 128:(u + 1) * 128] = True
        isA = isA[:nv]
        acc = m_raw[isA] - np.exp(RSCALE * dd[isA] - RSHIFT)
        m_raw[isA] = (np.log(np.maximum(acc, 1e-30)) + RSHIFT) / RSCALE
        md = m_raw * invn
        ddn = dd * invn
        d_an = np.sqrt(np.maximum(2.0 - 2.0 * md, 0.0))
        d_ap = np.sqrt(np.maximum(2.0 - 2.0 * ddn, 0.0))
        tri = np.maximum(d_ap - d_an + MARGIN, 0.0)
        np.add.at(per_sample_num, b_idx, tri)
        np.add.at(per_sample_den, b_idx, 1.0)
    return np.float32(np.mean(per_sample_num / per_sample_den))


def run_cores(inputs, label, pos_prot, trace=False, tmpdir=None):
    nt, in_maps, meta = _prep(inputs, label, pos_prot)
    nc, eng = _get_nc(nt)
    kw = {}
    if trace:
        kw = dict(trace=True, tmpdir=tmpdir)
    res = run_bass_kernel_spmd(nc, in_maps, core_ids=list(range(NCORES)), **kw)
    return res, nt, meta, eng


def kernel(inputs, label, pos_prot, only_update=0, **_unused):
    res, nt, meta, eng = run_cores(np.asarray(inputs), np.asarray(label),
                                   np.asarray(pos_prot))
    return _finish(res, nt, meta, eng)
